# revision 4
# baseline (speedup 1.0000x reference)
"""Conv2d 3x3 stride1 pad1 (B=32, C_in=128, C_out=256, H=W=56, fp32) on 8 TRN2
NeuronCores, data-parallel over batch (4 images/core), kernels+bias replicated.

Design:
  - Implicit GEMM: contraction dim = C_in = 128 = SBUF partition dim. For each
    (ky,kx) tap, out[co_tile, pix] += w_tap[ci, co_tile].T @ x_shift[ci, pix],
    9 taps accumulated in PSUM (start/stop flags).
  - Zero-padded image strip per batch element in SBUF ([128, 58*58]); the rhs
    of every matmul is a strided [128, 8, 56] slice (8 output rows) whose tap
    shift is just a flat offset ky*58+kx into the strip. N=448 <= one PSUM bank.
  - float32r (TF32) matmuls: 1 cycle/row for N>=256 (4x faster than fp32 path).
    Inputs are pre-rounded to TF32 on the host, which makes every product
    exact in fp32; PSUM accumulates fp32. End-to-end Frobenius rel err vs the
    fp32 reference ~2.8e-4 (pure input-rounding error).
  - Host pre-work: pad + transpose x to [ci, pix] strips, transpose kernels to
    [ci, (tap, co)] so all device DMAs are contiguous; TF32-round both.
  - DMA orchestration: input DMAs chunked (weights tap0 + first 11 rows of the
    first image land in ~1.5us so the PE starts early); input on the SP HWDGE
    ring, output on the Activation ring; bias-add fused into the PSUM->SBUF
    copy (alternating ScalarE activation / VectorE tensor_scalar_add); output
    DMA'd per 8-row group ([128, 448] contiguous).
"""
import sys
import numpy as np

try:
    import concourse.bacc as bacc
except ImportError:
    sys.path.insert(0, '/opt/trn_rl_repo')
    import concourse.bacc as bacc
import concourse.tile as tile
from concourse import mybir
from concourse.bass_utils import run_bass_kernel_spmd

N_CORES = 8
B, B_SH, CI, CO, H, W, K = 32, 4, 128, 256, 56, 56, 3
HP = H + 2
NPIX_PAD = HP * HP
TAPS = [(ky, kx) for ky in range(K) for kx in range(K)]
f32 = mybir.dt.float32
f32r = mybir.dt.float32r
RPT = 8                  # output rows per PSUM tile
N_RG = H // RPT          # 7 row groups
NVAL = RPT * W           # 448


def _tf32_round(a):
    u = np.ascontiguousarray(a, dtype=np.float32).view(np.uint32)
    lsb = (u >> 13) & 1
    u2 = (u + 0xFFF + lsb) & np.uint32(0xFFFFE000)
    return u2.view(np.float32)


def _build_nc(psum_bufs=8, ostage_bufs=6):
    nc = bacc.Bacc("TRN2", target_bir_lowering=False, debug=False)
    xp_d = nc.dram_tensor("xp", [B_SH, CI, NPIX_PAD], f32r, kind="ExternalInput")
    wt_d = nc.dram_tensor("wt", [CI, 9 * CO], f32r, kind="ExternalInput")
    b_d = nc.dram_tensor("bias", [CO], f32, kind="ExternalInput")
    o_d = nc.dram_tensor("out", [B_SH, CO, H, W], f32, kind="ExternalOutput")

    with tile.TileContext(nc) as tc:
        with tc.tile_pool(name="const", bufs=1) as cpool, \
             tc.tile_pool(name="ostage", bufs=ostage_bufs) as opool, \
             tc.tile_pool(name="psum", bufs=psum_bufs, space="PSUM") as ppool:

            xb = [cpool.tile([CI, NPIX_PAD], f32r, name=f"xb{b}")
                  for b in range(B_SH)]
            wr = cpool.tile([CI, 9 * CO], f32r)
            bsb = cpool.tile([128, 2], f32)

            # PE warmup: ~25 dummy matmuls on zeroed operands keep the PE busy
            # through the HAM/p-state ramp (~3.4us at 1.2GHz otherwise) while
            # the input DMAs land; result is never read. Costs ~50ns in the
            # cost-model schedule, saves ~1.5-3us of cold-clock matmuls on HW.
            wt_warm = cpool.tile([128, 64], f32, name="warm")
            nc.gpsimd.memset(wt_warm[:], 0.0)
            wps = ppool.tile([64, 64], f32, tag="ps")
            for _ in range(25):
                nc.tensor.matmul(wps[:], wt_warm[:, :64], wt_warm[:],
                                 start=True, stop=True)

            def dma_x_chunk(b, r):
                if r < N_RG:
                    lo, hi = r * RPT * HP, (r * RPT + RPT) * HP
                else:
                    lo, hi = H * HP, NPIX_PAD
                nc.sync.dma_start(xb[b][:, lo:hi], xp_d.ap()[b][:, lo:hi])

            # first matmul group needs w tap0 + x image0 rows 0..10; the rest
            # of image0 streams before taps 1-8 (PE consumes rows faster than
            # taps early on, and group0 is tap-gated anyway)
            nc.sync.dma_start(wr[:, 0:CO], wt_d.ap()[:, 0:CO])
            dma_x_chunk(0, 0)
            dma_x_chunk(0, 1)
            for r in range(2, N_RG + 1):
                dma_x_chunk(0, r)
            for t in range(1, 9):
                nc.sync.dma_start(wr[:, t * CO:(t + 1) * CO],
                                  wt_d.ap()[:, t * CO:(t + 1) * CO])
            nc.sync.dma_start(bsb[:], b_d.ap().rearrange("(t p) -> p t", p=128))
            for b in range(1, B_SH):
                for r in range(N_RG + 1):
                    dma_x_chunk(b, r)

            n_tile = 0
            for b in range(B_SH):
                xv = xb[b][:].rearrange("p (h w) -> p h w", h=HP)
                for ct in range(2):
                    for rg in range(N_RG):
                        ps = ppool.tile([128, NVAL], f32, tag="ps")
                        for t, (ky, kx) in enumerate(TAPS):
                            rhs = xv[:, rg * RPT + ky: rg * RPT + ky + RPT,
                                     kx:kx + W]
                            off = t * CO + ct * 128
                            nc.tensor.matmul(ps[:], wr[:, off:off + 128], rhs,
                                             start=(t == 0), stop=(t == 8))
                        ot = opool.tile([128, NVAL], f32, tag="ot")
                        if n_tile % 2 == 1:
                            nc.vector.tensor_scalar_add(ot[:], ps[:],
                                                        bsb[:, ct:ct + 1])
                        else:
                            nc.scalar.activation(
                                ot[:], ps[:],
                                mybir.ActivationFunctionType.Identity,
                                bias=bsb[:, ct:ct + 1])
                        nc.scalar.dma_start(
                            o_d.ap()[b, ct * 128:(ct + 1) * 128,
                                     rg * RPT:(rg + 1) * RPT, :]
                            .rearrange("c h w -> c (h w)"), ot[:])
                        n_tile += 1
    nc.compile()
    return nc


def _make_in_maps(x, kernels, bias):
    wt = _tf32_round(np.ascontiguousarray(
        kernels.reshape(CO, CI, 9).transpose(1, 2, 0)).reshape(CI, 9 * CO))
    bias = np.ascontiguousarray(bias, dtype=np.float32)
    in_maps = []
    for c in range(N_CORES):
        xs = x[c * B_SH:(c + 1) * B_SH]
        xp = np.zeros((B_SH, CI, HP, HP), np.float32)
        xp[:, :, 1:H + 1, 1:W + 1] = _tf32_round(xs)
        in_maps.append({"xp": xp.reshape(B_SH, CI, NPIX_PAD),
                        "wt": wt, "bias": bias})
    return in_maps


_NC_CACHE = []


def kernel(x, kernels, bias):
    x = np.ascontiguousarray(np.asarray(x), dtype=np.float32)
    kernels = np.ascontiguousarray(np.asarray(kernels), dtype=np.float32)
    bias = np.ascontiguousarray(np.asarray(bias), dtype=np.float32)
    if not _NC_CACHE:
        _NC_CACHE.append(_build_nc())
    nc = _NC_CACHE[0]
    in_maps = _make_in_maps(x, kernels, bias)
    res = run_bass_kernel_spmd(nc, in_maps, core_ids=list(range(N_CORES)))
    return np.concatenate([r["out"] for r in res.results], axis=0)
